# revision 58
# baseline (speedup 1.0000x reference)
"""Trainium2 Bass kernel for a GPT-style transformer block.

B=4, T=2048, C=1024, H=16 heads (D=64), FF=4096.
Sharding: 8 NeuronCores, core c = 2*b + h handles batch b, token half h
(queries/output tokens [h*1024, (h+1)*1024)); K/V are computed on-core over
the full sequence. One uniform SPMD program; per-core causality enters only
through data (host-rotated x and DMA'd multiplicative masks).

All matmul operands are bf16 (PSUM accumulation stays f32): bf16 enables
Fast Weight Load (4x faster LDWEIGHTS), halves HBM traffic and doubles DVE
throughput for element-wise work.  Activations stay channel-major
([channel, token]): LN statistics are taken over the partition dim with
ones-matmuls, rstd is computed as exp(-0.5*ln(var+eps)) so ScalarE never
leaves the exp/ln table set during attention, softmax runs without max
subtraction (scores are bounded), and V is augmented with a ones column so
the softmax denominator falls out of the AV matmul itself.

PSUM budget: LN-stats (2 banks) + QKV accumulators (6) run first; then the
attention region holds psS 2x[128,1024] (4) + per-head psY 2x[65,512] (2)
alongside a shared 2x[128,512] GEMM-accumulator pool (2) used by
out-projection, LN2 stats and the MLP, so chunk-0 MLP matmuls overlap
chunk-1 attention's exp-bound pipeline.
"""
import sys

sys.path.insert(0, "/opt/trn_rl_repo")

import numpy as np
import ml_dtypes
from contextlib import ExitStack

import concourse.bass as bass
import concourse.tile as tile
from concourse import bacc, mybir

F32 = mybir.dt.float32
BF16 = mybir.dt.bfloat16
AF = mybir.ActivationFunctionType
OP = mybir.AluOpType

B, T, C, H, D = 4, 2048, 1024, 16, 64
FF = 4 * C
TOK = T // 2          # tokens owned per core
NCB = C // 128        # 8 channel blocks
NFB = FF // 128       # 32 ff blocks
NPAIR = H // 2        # 8 head pairs (128 channels each)
NSB = T // 128        # 16 s-blocks
NV = 8                # diagonal mask visits (4 per q-chunk)
TRIPS = (12, 16)      # key-blocks visited per 512-query chunk

_CACHE = {}


def _build():
    nc = bacc.Bacc("TRN2", target_bir_lowering=False, debug=False, num_devices=8)

    xb_d = nc.dram_tensor("xTb", [C, T], BF16, kind="ExternalInput").ap()
    xres_d = nc.dram_tensor("xres", [C, TOK], F32, kind="ExternalInput").ap()
    wqk_d = nc.dram_tensor("wqk", [2 * NPAIR, 128, NCB, 128], BF16,
                           kind="ExternalInput").ap()
    wv_d = nc.dram_tensor("wv", [128, NCB, 1024], BF16, kind="ExternalInput").ap()
    wo_d = nc.dram_tensor("wo", [NCB, 128, NCB, 128], BF16, kind="ExternalInput").ap()
    wfc_d = nc.dram_tensor("wfc", [NFB, 128, NCB, 128], BF16,
                           kind="ExternalInput").ap()
    wproj_d = nc.dram_tensor("wproj", [NCB, 4, 128, NFB // 4, 128], BF16,
                             kind="ExternalInput").ap()
    mask_d = nc.dram_tensor("masks", [128, NV, 512], BF16, kind="ExternalInput").ap()
    vmask_d = nc.dram_tensor("vmask", [128, NSB], F32, kind="ExternalInput").ap()
    out_d = nc.dram_tensor("outT", [C, TOK], F32, kind="ExternalOutput").ap()

    with tile.TileContext(nc) as tc, ExitStack() as top:
        persist = top.enter_context(tc.tile_pool(name="persist", bufs=1))
        ones_f = persist.tile([128, 1], F32)
        nc.vector.memset(ones_f[:], 1.0)
        ones_b = persist.tile([128, 1], BF16)
        nc.vector.tensor_copy(ones_b[:], ones_f[:])
        eps_t = persist.tile([128, 1], F32)
        nc.vector.memset(eps_t[:], 1e-5)

        # big SBUF tensors that persist from QKV into attention
        pQK = top.enter_context(tc.tile_pool(name="pQK", bufs=1))
        kT = pQK.tile([128, NPAIR, T], BF16)
        qT = pQK.tile([128, NPAIR, TOK], BF16)
        v_aug = pQK.tile([128, NSB, H, 65], BF16)
        vmask = pQK.tile([128, NSB], F32)
        nc.sync.dma_start(vmask[:], vmask_d)

        # ============ Phase 1: LN1 + QKV (single fused pass) ============
        with nc.named_scope("ln1_qkv"), \
             tc.tile_pool(name="pxb", bufs=2) as pxb, \
             tc.tile_pool(name="pst", bufs=2) as pst, \
             tc.tile_pool(name="px2", bufs=3) as px2, \
             tc.tile_pool(name="pbw", bufs=2) as pbw, \
             tc.tile_pool(name="pbv", bufs=1) as pbv, \
             tc.tile_pool(name="pln", bufs=4) as pln, \
             tc.tile_pool(name="psStat", bufs=1, space="PSUM") as psStat, \
             tc.tile_pool(name="psB", bufs=2, space="PSUM") as psB:
            vw = pbv.tile([128, NCB, 1024], BF16)
            nc.sync.dma_start(vw[:], wv_d)
            ln_halves = []
            for tck in range(2):
                # ---- LN1 over two 512-token halves of this chunk ----
                for half in range(2):
                    gsl = slice(tck * 1024 + half * 512, tck * 1024 + (half + 1) * 512)
                    xbh = pxb.tile([128, NCB, 512], BF16, tag="xb")
                    for cb in range(NCB):
                        nc.sync.dma_start(xbh[:, cb, :],
                                          xb_d[cb * 128:(cb + 1) * 128, gsl])
                    sum_ps = psStat.tile([1, 512], F32, tag="sum")
                    sq_ps = psStat.tile([1, 512], F32, tag="sq")
                    for cb in range(NCB):
                        x2h = px2.tile([128, 512], BF16, tag="x2")
                        nc.vector.tensor_tensor(x2h[:], xbh[:, cb, :], xbh[:, cb, :],
                                                OP.mult)
                        nc.tensor.matmul(sum_ps[:], ones_b[:, 0:1], xbh[:, cb, :],
                                         start=(cb == 0), stop=(cb == NCB - 1))
                        nc.tensor.matmul(sq_ps[:], ones_b[:, 0:1], x2h[:],
                                         start=(cb == 0), stop=(cb == NCB - 1))
                    mean_v = pst.tile([1, 512], F32, tag="mean")
                    rstd_v = pst.tile([1, 512], F32, tag="rstd")
                    nc.scalar.mul(mean_v[:], sum_ps[:], 1.0 / C)
                    nc.scalar.mul(rstd_v[:], sq_ps[:], 1.0 / C)
                    msq = px2.tile([1, 512], F32, tag="msq")
                    nc.vector.tensor_mul(msq[:], mean_v[:], mean_v[:])
                    nc.vector.tensor_sub(rstd_v[:], rstd_v[:], msq[:])
                    # rstd = exp(-0.5*ln(var+eps)): stays in the exp/ln ACT
                    # table set so attention's exp never triggers a reload.
                    nc.scalar.activation(rstd_v[:], rstd_v[:], AF.Ln,
                                         bias=eps_t[0:1, :])
                    nc.scalar.activation(rstd_v[:], rstd_v[:], AF.Exp, scale=-0.5)
                    mean_b = pst.tile([1, 512], BF16, tag="meanb")
                    rstd_b = pst.tile([1, 512], BF16, tag="rstdb")
                    nc.vector.tensor_copy(mean_b[:], mean_v[:])
                    nc.vector.tensor_copy(rstd_b[:], rstd_v[:])
                    mb = pst.tile([128, 512], BF16, tag="mb")
                    rb = pst.tile([128, 512], BF16, tag="rb")
                    nc.gpsimd.partition_broadcast(mb[:], mean_b[:])
                    nc.gpsimd.partition_broadcast(rb[:], rstd_b[:])
                    lnh = pln.tile([128, NCB, 512], BF16, tag="ln")
                    for cb in range(NCB):
                        xc = px2.tile([128, 512], BF16, tag="xc")
                        nc.vector.tensor_sub(xc[:], xbh[:, cb, :], mb[:])
                        nc.vector.tensor_tensor(lnh[:, cb, :], xc[:], rb[:], OP.mult)
                    ln_halves.append(lnh)

                # ---- QKV matmuls for this 1024-token chunk ----
                # K for every chunk; Q only for own tokens (chunk 1).
                ha, hb = ln_halves[2 * tck], ln_halves[2 * tck + 1]
                tsl = slice(tck * 1024, (tck + 1) * 1024)
                ocb_list = (list(range(NPAIR, 2 * NPAIR)) if tck == 0
                            else list(range(2 * NPAIR)))
                for ocb in ocb_list:
                    is_q = ocb < NPAIR
                    pblk = ocb % NPAIR
                    wt = pbw.tile([128, NCB, 128], BF16, tag="wt")
                    nc.sync.dma_start(wt[:], wqk_d[ocb])
                    acc = psB.tile([128, 1024], F32, tag="qk")
                    for cb in range(NCB):
                        for n2, lh in ((0, ha), (1, hb)):
                            nc.tensor.matmul(acc[:, n2 * 512:(n2 + 1) * 512],
                                             wt[:, cb, :], lh[:, cb, :],
                                             start=(cb == 0), stop=(cb == NCB - 1))
                    if is_q:
                        nc.scalar.copy(qT[:, pblk, :], acc[:])
                    else:
                        nc.scalar.copy(kT[:, pblk, tsl], acc[:])
                for sb_l in range(8):
                    sblk = tck * 8 + sb_l
                    lh = ha if sb_l < 4 else hb
                    bsl = slice((sb_l % 4) * 128, (sb_l % 4 + 1) * 128)
                    # vh inner so each lnh weight-load serves both v halves
                    vps0 = psB.tile([128, 512], F32, tag="vps")
                    vps1 = psB.tile([128, 512], F32, tag="vps")
                    for cb in range(NCB):
                        nc.tensor.matmul(vps0[:], lh[:, cb, bsl], vw[:, cb, 0:512],
                                         start=(cb == 0), stop=(cb == NCB - 1))
                        nc.tensor.matmul(vps1[:], lh[:, cb, bsl], vw[:, cb, 512:1024],
                                         start=(cb == 0), stop=(cb == NCB - 1))
                    nc.vector.tensor_scalar_mul(v_aug[:, sblk, 0:8, 0:64], vps0[:],
                                                vmask[:, sblk:sblk + 1])
                    nc.vector.tensor_scalar_mul(v_aug[:, sblk, 8:16, 0:64], vps1[:],
                                                vmask[:, sblk:sblk + 1])
                    nc.vector.tensor_copy(
                        v_aug[:, sblk, :, 64:65],
                        vmask[:, sblk:sblk + 1].broadcast_to([128, H, 1]))

        # ===== Phase 2: attention + out-proj + LN2 + MLP, pipelined by qc =====
        with tc.tile_pool(name="pY", bufs=1) as pY, \
             tc.tile_pool(name="pct", bufs=3) as pct, \
             tc.tile_pool(name="pcn", bufs=1) as pcn, \
             tc.tile_pool(name="pres", bufs=2) as pres, \
             tc.tile_pool(name="pl2", bufs=1) as pl2, \
             tc.tile_pool(name="pfw", bufs=2) as pfw, \
             tc.tile_pool(name="ppw", bufs=2) as ppw, \
             tc.tile_pool(name="ph", bufs=1) as ph, \
             tc.tile_pool(name="pof", bufs=1) as pof, \
             tc.tile_pool(name="psS", bufs=2, space="PSUM") as psS, \
             tc.tile_pool(name="psY", bufs=2, space="PSUM") as psY, \
             tc.tile_pool(name="psM", bufs=2, space="PSUM") as psM:
            masks = pY.tile([128, NV, 512], BF16)
            nc.sync.dma_start(masks[:], mask_d)
            y_sb = pY.tile([128, NPAIR, TOK], BF16)
            x1 = pY.tile([128, NCB, TOK], BF16)
            ln2_tiles = {}

            def attn_chunk(qc):
                qsl = slice(qc * 512, (qc + 1) * 512)
                trip = TRIPS[qc]
                with nc.named_scope(f"attn_qc{qc}"):
                    for pair in range(NPAIR):
                        y_a = psY.tile([65, 512], F32, tag="psY")
                        y_b = psY.tile([65, 512], F32, tag="psY")
                        for j in range(trip):
                            st = (j == 0)
                            sp = (j == trip - 1)
                            jsl = slice(j * 128, (j + 1) * 128)
                            s01 = psS.tile([128, 1024], F32, tag="s01")
                            nc.tensor.matmul(s01[:, 0:512], kT[0:64, pair, jsl],
                                             qT[0:64, pair, qsl], start=True,
                                             stop=True, tile_position=(0, 0))
                            nc.tensor.matmul(s01[:, 512:1024], kT[64:128, pair, jsl],
                                             qT[64:128, pair, qsl], start=True,
                                             stop=True, tile_position=(64, 0))
                            p01r = pct.tile([128, 1024], BF16, tag="p01r", bufs=2)
                            if j >= trip - 4:  # diagonal: mask needed
                                p01 = pct.tile([128, 1024], BF16, tag="p01", bufs=2)
                                nc.scalar.activation(p01[:], s01[:], AF.Exp,
                                                     scale=0.125)
                                vi = qc * 4 + (j - (trip - 4))
                                m2 = masks[:, vi:vi + 1, :].broadcast_to([128, 2, 512])
                                nc.vector.tensor_tensor(p01r[:], p01[:], m2, OP.mult)
                            else:
                                nc.scalar.activation(p01r[:], s01[:], AF.Exp,
                                                     scale=0.125)
                            nc.tensor.matmul(y_a[:], v_aug[:, j, 2 * pair, :],
                                             p01r[:, 0:512], start=st, stop=sp)
                            nc.tensor.matmul(y_b[:], v_aug[:, j, 2 * pair + 1, :],
                                             p01r[:, 512:1024], start=st, stop=sp)
                        for hh, y_h in ((0, y_a), (1, y_b)):
                            # Evacuate PSUM immediately (frees the bank for the
                            # next pair), normalize from SBUF off the PE path.
                            ysr = pcn.tile([65, 512], BF16, tag="ysr", bufs=2)
                            nc.vector.tensor_copy(ysr[:], y_h[:])
                            rec = pcn.tile([1, 512], F32, tag="rec", bufs=2)
                            nc.vector.reciprocal(rec[:], ysr[64:65, :])
                            rbc = pcn.tile([128, 512], F32, tag="rbc", bufs=2)
                            nc.gpsimd.partition_broadcast(rbc[:], rec[:])
                            nc.vector.tensor_tensor(
                                y_sb[hh * 64:(hh + 1) * 64, pair, qsl],
                                ysr[0:64, :], rbc[0:64, :], OP.mult)

            def oproj_ln2_chunk(qc):
                qsl = slice(qc * 512, (qc + 1) * 512)
                with nc.named_scope(f"oproj_ln2_qc{qc}"):
                    for ocb in range(NCB):
                        wt = pres.tile([128, NCB, 128], BF16, tag="wo")
                        nc.sync.dma_start(wt[:], wo_d[ocb])
                        acc = psM.tile([128, 512], F32, tag="macc")
                        for cb in range(NCB):
                            nc.tensor.matmul(acc[:], wt[:, cb, :], y_sb[:, cb, qsl],
                                             start=(cb == 0), stop=(cb == NCB - 1))
                        xf = pres.tile([128, 512], F32, tag="xres")
                        nc.sync.dma_start(xf[:], xres_d[ocb * 128:(ocb + 1) * 128, qsl])
                        nc.vector.tensor_add(x1[:, ocb, qsl], acc[:], xf[:])
                    # LN2 stats over channels for this 512-token chunk
                    sum_ps = psM.tile([1, 512], F32, tag="macc")
                    sq_ps = psM.tile([1, 512], F32, tag="macc")
                    for cb in range(NCB):
                        x2 = pcn.tile([128, 512], BF16, tag="x2b", bufs=1)
                        nc.vector.tensor_tensor(x2[:], x1[:, cb, qsl], x1[:, cb, qsl],
                                                OP.mult)
                        nc.tensor.matmul(sum_ps[:], ones_b[:, 0:1], x1[:, cb, qsl],
                                         start=(cb == 0), stop=(cb == NCB - 1))
                        nc.tensor.matmul(sq_ps[:], ones_b[:, 0:1], x2[:],
                                         start=(cb == 0), stop=(cb == NCB - 1))
                    m2v = pcn.tile([1, 512], F32, tag="m2v")
                    v2 = pcn.tile([1, 512], F32, tag="v2")
                    nc.scalar.mul(m2v[:], sum_ps[:], 1.0 / C)
                    nc.scalar.mul(v2[:], sq_ps[:], 1.0 / C)
                    msq = pcn.tile([1, 512], F32, tag="rec", bufs=2)
                    nc.vector.tensor_mul(msq[:], m2v[:], m2v[:])
                    nc.vector.tensor_sub(v2[:], v2[:], msq[:])
                    nc.scalar.activation(v2[:], v2[:], AF.Ln, bias=eps_t[0:1, :])
                    nc.scalar.activation(v2[:], v2[:], AF.Exp, scale=-0.5)
                    m2b = pcn.tile([1, 512], BF16, tag="m2b")
                    r2b = pcn.tile([1, 512], BF16, tag="r2b")
                    nc.vector.tensor_copy(m2b[:], m2v[:])
                    nc.vector.tensor_copy(r2b[:], v2[:])
                    mb2 = pcn.tile([128, 512], BF16, tag="mb2")
                    rb2 = pcn.tile([128, 512], BF16, tag="rb2")
                    nc.gpsimd.partition_broadcast(mb2[:], m2b[:])
                    nc.gpsimd.partition_broadcast(rb2[:], r2b[:])
                    ln2r = pl2.tile([128, NCB, 512], BF16, tag="ln2")
                    ln2_tiles[qc] = ln2r
                    for cb in range(NCB):
                        xc = pcn.tile([128, 512], BF16, tag="xc2", bufs=1)
                        nc.vector.tensor_sub(xc[:], x1[:, cb, qsl], mb2[:])
                        nc.vector.tensor_tensor(ln2r[:, cb, :], xc[:], rb2[:],
                                                OP.mult)

            def mlp_chunk(qc):
                qsl = slice(qc * 512, (qc + 1) * 512)
                ln2r = ln2_tiles[qc]
                with nc.named_scope(f"mlp_qc{qc}"):
                    h_r = ph.tile([128, NFB, 512], BF16, tag="h")
                    for fb in range(NFB):
                        wt = pfw.tile([128, NCB, 128], BF16, tag="fwt")
                        nc.sync.dma_start(wt[:], wfc_d[fb])
                        fc = psM.tile([128, 512], F32, tag="macc")
                        for cb in range(NCB):
                            nc.tensor.matmul(fc[:], wt[:, cb, :], ln2r[:, cb, :],
                                             start=(cb == 0), stop=(cb == NCB - 1))
                        nc.scalar.activation(h_r[:, fb, :], fc[:], AF.Gelu)
                    for ocb in range(NCB):
                        acc = psM.tile([128, 512], F32, tag="macc")
                        for fh in range(4):
                            wt = ppw.tile([128, NFB // 4, 128], BF16, tag="pwt")
                            nc.sync.dma_start(wt[:], wproj_d[ocb, fh])
                            for fi in range(NFB // 4):
                                fb = fh * (NFB // 4) + fi
                                nc.tensor.matmul(acc[:], wt[:, fi, :], h_r[:, fb, :],
                                                 start=(fb == 0), stop=(fb == NFB - 1))
                        of = pof.tile([128, 512], F32, tag="of")
                        nc.vector.tensor_add(of[:], acc[:], x1[:, ocb, qsl])
                        nc.sync.dma_start(out_d[ocb * 128:(ocb + 1) * 128, qsl], of[:])

            attn_chunk(0)
            oproj_ln2_chunk(0)
            attn_chunk(1)
            mlp_chunk(0)
            oproj_ln2_chunk(1)
            mlp_chunk(1)

    nc.compile()
    return nc


def _prep_weights(g1, w_qkv, w_o, g2, w_fc, w_proj):
    bf = ml_dtypes.bfloat16
    g1 = np.asarray(g1, np.float32)
    g2 = np.asarray(g2, np.float32)
    wqkvT = np.ascontiguousarray((np.asarray(w_qkv, np.float32) * g1[None, :]).T)
    woT = np.ascontiguousarray(np.asarray(w_o, np.float32).T)
    wfcT = np.ascontiguousarray((np.asarray(w_fc, np.float32) * g2[None, :]).T)
    wprojT = np.ascontiguousarray(np.asarray(w_proj, np.float32).T)

    # wqk[ocb, r, cb, f]: ocb 0..7 = Q pair blocks, 8..15 = K pair blocks
    wqk = np.empty((2 * NPAIR, 128, NCB, 128), np.float32)
    for ocb in range(2 * NPAIR):
        col0 = (0 if ocb < NPAIR else C) + (ocb % NPAIR) * 128
        blk = wqkvT[:, col0:col0 + 128].reshape(NCB, 128, 128)  # [cb, r, f]
        wqk[ocb] = blk.transpose(1, 0, 2)
    wv = wqkvT[:, 2 * C:3 * C].reshape(NCB, 128, 1024).transpose(1, 0, 2)
    wo = np.empty((NCB, 128, NCB, 128), np.float32)
    for ocb in range(NCB):
        blk = woT[:, ocb * 128:(ocb + 1) * 128].reshape(NCB, 128, 128)
        wo[ocb] = blk.transpose(1, 0, 2)
    wfc = np.empty((NFB, 128, NCB, 128), np.float32)
    for fb in range(NFB):
        blk = wfcT[:, fb * 128:(fb + 1) * 128].reshape(NCB, 128, 128)
        wfc[fb] = blk.transpose(1, 0, 2)
    wproj = np.empty((NCB, 4, 128, NFB // 4, 128), np.float32)
    for ocb in range(NCB):
        blk = wprojT[:, ocb * 128:(ocb + 1) * 128].reshape(NFB, 128, 128)
        for fh in range(4):
            wproj[ocb, fh] = blk[fh * (NFB // 4):(fh + 1) * (NFB // 4)].transpose(1, 0, 2)
    return {"wqk": wqk.astype(bf), "wv": np.ascontiguousarray(wv).astype(bf),
            "wo": wo.astype(bf), "wfc": wfc.astype(bf), "wproj": wproj.astype(bf)}


def _prep(x, g1, w_qkv, w_o, g2, w_fc, w_proj):
    """Build the 8 per-core input maps (all host-side)."""
    bf = ml_dtypes.bfloat16
    x = np.asarray(x, np.float32)
    wmap = _prep_weights(g1, w_qkv, w_o, g2, w_fc, w_proj)

    in_maps = []
    for c in range(8):
        b, h = c // 2, c % 2
        # Rotate the sequence so the core's own tokens are always chunk 1
        # ([TOK:T]) of xT: h=0 swaps halves, h=1 keeps order.
        xb = x[b]
        if h == 0:
            xb = np.concatenate([xb[TOK:], xb[:TOK]], axis=0)
        xT = np.ascontiguousarray(xb.T)
        # Causal masks in ROTATED key coordinates, packed [row, visit, 512].
        rot = (np.arange(T) + (TOK if h == 0 else 0)) % T
        gq = rot[TOK:]           # global positions of own (query) tokens
        gk = rot                 # global positions of keys in rotated order
        msk = np.zeros((NV, 128, 512), np.float32)
        for qc in range(2):
            trip = TRIPS[qc]
            qpos = gq[qc * 512:(qc + 1) * 512]
            for i in range(4):
                j = trip - 4 + i
                kpos = gk[j * 128:(j + 1) * 128]
                msk[qc * 4 + i] = (kpos[:, None] <= qpos[None, :])
        msk = np.ascontiguousarray(msk.transpose(1, 0, 2))  # [128, NV, 512]
        # vmask: zero K/V rows never visible to any own query
        vmask = (gk <= gq.max()).astype(np.float32).reshape(NSB, 128).T
        vmask = np.ascontiguousarray(vmask)  # [128, NSB]
        in_maps.append({"xTb": xT.astype(bf),
                        "xres": np.ascontiguousarray(xT[:, TOK:T]),
                        "masks": msk.astype(bf),
                        "vmask": vmask, **wmap})
    return in_maps


def kernel(x, g1, w_qkv, w_o, g2, w_fc, w_proj, _trace=False, **_tk):
    from concourse.bass_utils import run_bass_kernel_spmd
    if "nc" not in _CACHE:
        _CACHE["nc"] = _build()
    nc = _CACHE["nc"]
    in_maps = _prep(x, g1, w_qkv, w_o, g2, w_fc, w_proj)
    res = run_bass_kernel_spmd(nc, in_maps, core_ids=list(range(8)),
                               trace=_trace, **_tk)
    _CACHE["last"] = res
    out = np.empty((B, T, C), np.float32)
    for c in range(8):
        b, h = c // 2, c % 2
        out[b, h * TOK:(h + 1) * TOK, :] = res.results[c]["outT"].T
    return out
